# revision 1
# baseline (speedup 1.0000x reference)
"""DeformableConv1d Trainium2 kernel.

Problem: N=16, C_in=64, L=8192, K=3, C_out=64, PAD=1.
Sharding: data-parallel over batch; each of 8 cores handles 2 samples.

Math (validated against the jax reference):
  offsets = conv1d(x, w_off, pad=1) + b_off      (only channels 0,2,4 used)
  grid[l',k] = clip(l'+1 + off_k[l'], 0, 8193)   (padded coords)
  left = floor(grid), alpha = grid - left
  out[n, o, q*64+r] = sum_{k,t} w[o, k*64+t] * xd[n, r, t*128+q, k]
  xd[n, c, l', k] = (1-a)*xp[n, c, left] + a*xp[n, c, left+1]
where l = q*64+r (q in [0,128), r in [0,64)), t in [0,64).

Kernel structure per core:
  P1: load x -> SBUF; offsets conv on PE; PE-transpose x -> XT2[2*8195, 64]
      fp32 DRAM (row = padded position, all 64 channels); compute
      idx/alpha tiles [128,128] (partition p = 64n+t, free q).
  P2: for each output quarter Q (2048 cols) and tap k: indirect-DMA gather
      of row pairs (left, left+1) -> GB[128, 32, 128]; DVE blend
      D = R-L, E = alpha*D; PE matmuls acc out += WkT @ L + WkT @ E into
      PSUM [64, 2048] per sample; ACT drain (+bias); DMA out.
"""

import numpy as np

N, C, L, K, PAD = 16, 64, 8192, 3, 1
NS = 2                 # samples per core
NCORES = 8
LP = L + 2 * PAD       # 8194
XROWS = LP + 1         # 8195 rows per sample (incl. overflow row for left+1)
CO = 64
NQ = 4                 # output quarters
QW = L // NQ           # 2048 output cols per quarter
QB = QW // 64          # 32 q-values per quarter

_CACHE = {}


def _build_nc(debug=False, reps=1, stage='full'):
    import concourse.bass as bass
    import concourse.tile as tile
    from concourse import bacc, mybir
    from concourse.bass import IndirectOffsetOnAxis

    f32 = mybir.dt.float32
    i32 = mybir.dt.int32
    i16 = mybir.dt.int16
    Alu = mybir.AluOpType
    Act = mybir.ActivationFunctionType

    nc = bacc.Bacc("TRN2", target_bir_lowering=False)

    xin = nc.dram_tensor("xin", [NS, C, L], f32, kind="ExternalInput")
    woffT = nc.dram_tensor("woffT", [C, 9], f32, kind="ExternalInput")
    wTk2 = nc.dram_tensor("wTk2", [128, 192], f32, kind="ExternalInput")
    boff = nc.dram_tensor("boff", [3, 1], f32, kind="ExternalInput")
    bout = nc.dram_tensor("bout", [CO, 1], f32, kind="ExternalInput")
    base128 = nc.dram_tensor("base128", [128, 128], f32, kind="ExternalInput")
    row128 = nc.dram_tensor("row128", [128, 128], f32, kind="ExternalInput")
    ident = nc.dram_tensor("ident", [64, 64], f32, kind="ExternalInput")
    out = nc.dram_tensor("out", [NS, CO, L], f32, kind="ExternalOutput")

    if debug:
        xt2 = nc.dram_tensor("xt2", [NS * XROWS, C], f32, kind="ExternalOutput")
        d_offs = nc.dram_tensor("d_offs", [NS, 3, L], f32, kind="ExternalOutput")
        d_idx = nc.dram_tensor("d_idx", [K, 128, 128], i16, kind="ExternalOutput")
        d_alpha = nc.dram_tensor("d_alpha", [K, 128, 128], f32, kind="ExternalOutput")
        d_gb = nc.dram_tensor("d_gb", [128, QB, 128], f32, kind="ExternalOutput")
    else:
        xt2 = nc.dram_tensor("xt2", [NS * XROWS, C], f32)  # internal scratch
    idxd = nc.dram_tensor("idxd", [K, 16, 1024], mybir.dt.int16)  # wrapped idx bounce

    with tile.TileContext(nc) as tc:
      for rep in range(reps):
        with tc.tile_pool(name=f"const{rep}", bufs=1) as constp:
            woffT_t = constp.tile([C, 9], f32)
            nc.sync.dma_start(woffT_t[:], woffT[:])
            wTk2_t = constp.tile([128, 192], f32)
            nc.sync.dma_start(wTk2_t[:], wTk2[:])
            boff_t = constp.tile([3, 1], f32)
            nc.sync.dma_start(boff_t[:], boff[:])
            bout_t = constp.tile([CO, 1], f32)
            nc.sync.dma_start(bout_t[:], bout[:])
            base_t = constp.tile([128, 128], f32)
            nc.sync.dma_start(base_t[:], base128[:])
            row_t = constp.tile([128, 128], f32)
            nc.sync.dma_start(row_t[:], row128[:])
            id_t = constp.tile([64, 64], f32)
            nc.sync.dma_start(id_t[:], ident[:])
            zrow = constp.tile([1, C], f32)
            nc.vector.memset(zrow[:], 0.0)
            # zero pad rows of xt2 (rows 0, 8193, 8194 per sample)
            for n in range(NS):
                for r in (0, LP - 1, LP):
                    nc.sync.dma_start(xt2[n * XROWS + r : n * XROWS + r + 1, :],
                                      zrow[:1, :])

            alpha_t = [constp.tile([128, 128], f32, tag=f"alpha{k}", name=f"al{rep}_{k}") for k in range(K)]
            idx_t = [constp.tile([128, 128], i16, tag=f"idx{k}", name=f"ix{rep}_{k}") for k in range(K)]
            # wrapped+replicated gather index tiles for dma_gather (one per (k, Q)):
            # wkq[k][Q][16g+a, ql*8 + 4n + h] = XROWS*n + left_k[(16h+a)*128 + Q*32 + ql]
            wkq_t = [[constp.tile([128, 8 * QB], i16, tag=f"wk{k}_{Q}", name=f"wk{rep}_{k}_{Q}")
                      for Q in range(NQ)] for k in range(K)]

            # ---------------- phase 1 ----------------
            with tc.tile_pool(name=f"xp{rep}", bufs=2) as xpp, \
                 tc.tile_pool(name=f"offs{rep}", bufs=2) as offsp, \
                 tc.tile_pool(name=f"cpsum{rep}", bufs=1, space="PSUM") as cpsump, \
                 tc.tile_pool(name=f"tpsum{rep}", bufs=3, space="PSUM") as tpsump, \
                 tc.tile_pool(name=f"stage{rep}", bufs=3) as stagep, \
                 tc.tile_pool(name=f"small{rep}", bufs=2) as smallp:

                offs_tiles = []
                for n in range(NS):
                    xp = xpp.tile([C, LP], f32, tag="xp")
                    nc.vector.memset(xp[:, 0:1], 0.0)
                    nc.vector.memset(xp[:, LP - 1 : LP], 0.0)
                    nc.sync.dma_start(xp[:, 1 : 1 + L], xin[n])

                    # offsets conv: off[k, l'] = sum_c sum_j woff[k,c,j]*xp[c, l'+j]
                    offs_n = offsp.tile([3, L], f32, tag="offs")
                    offs_tiles.append(offs_n)
                    for c2 in range(L // 2048):
                        cps = cpsump.tile([3, 2048], f32, tag="cps")
                        for b in range(4):
                            col0 = c2 * 2048 + b * 512
                            for j in range(3):
                                nc.tensor.matmul(
                                    cps[:, b * 512 : (b + 1) * 512],
                                    lhsT=woffT_t[:, j * 3 : (j + 1) * 3],
                                    rhs=xp[:, j + col0 : j + col0 + 512],
                                    start=(j == 0), stop=(j == 2),
                                )
                        nc.scalar.activation(offs_n[:, c2 * 2048 : (c2 + 1) * 2048],
                                             cps[:], Act.Identity, bias=boff_t[:])

                    # transpose x into xt2 rows (row l+1 <- x[:, l])
                    for g in range(8):
                        l0 = g * 1024
                        tps = tpsump.tile([128, 512], f32, tag="tps")
                        for j in range(8):
                            nc.tensor.transpose(
                                tps[:, j * 64 : (j + 1) * 64],
                                xp[:, 1 + l0 + j * 128 : 1 + l0 + (j + 1) * 128],
                                id_t[:],
                            )
                        st = stagep.tile([128, 512], f32, tag="st")
                        nc.vector.tensor_copy(st[:], tps[:])
                        nc.sync.dma_start(
                            xt2[n * XROWS + 1 + l0 : n * XROWS + 1 + l0 + 1024, :]
                            .rearrange("(j p) c -> p j c", p=128),
                            st[:].rearrange("p (j c) -> p j c", c=64),
                        )

                # idx / alpha per tap k, in [p=64n+t, q] layout
                for k in range(K):
                    off128 = smallp.tile([128, 128], f32, tag="off128")
                    for n in range(NS):
                        nc.sync.dma_start(off128[n * 64 : (n + 1) * 64, :],
                                          offs_tiles[n][k : k + 1, :])
                    grid = smallp.tile([128, 128], f32, tag="grid")
                    nc.vector.tensor_tensor(grid[:], off128[:], base_t[:], op=Alu.add)
                    gridc = smallp.tile([128, 128], f32, tag="gridc")
                    nc.vector.tensor_scalar(gridc[:], grid[:], 0.0, float(LP - 1),
                                            op0=Alu.max, op1=Alu.min)
                    # floor(gridc), robust to cast rounding mode:
                    # c = cast(gridc); floor = c - (c > gridc)
                    casti = smallp.tile([128, 128], i32, tag="casti")
                    nc.vector.tensor_copy(casti[:], gridc[:])
                    castf = smallp.tile([128, 128], f32, tag="castf")
                    nc.vector.tensor_copy(castf[:], casti[:])
                    over = smallp.tile([128, 128], f32, tag="over")
                    nc.vector.tensor_tensor(over[:], castf[:], gridc[:], op=Alu.is_gt)
                    leftf = smallp.tile([128, 128], f32, tag="leftf")
                    nc.vector.tensor_tensor(leftf[:], castf[:], over[:],
                                            op=Alu.subtract)
                    nc.vector.tensor_tensor(alpha_t[k][:], gridc[:], leftf[:],
                                            op=Alu.subtract)
                    idxf = smallp.tile([128, 128], f32, tag="idxf")
                    nc.vector.tensor_tensor(idxf[:], leftf[:], row_t[:], op=Alu.add)
                    nc.vector.tensor_copy(idx_t[k][:], idxf[:])
                    # bounce to DRAM in wrapped layout:
                    # idxd[k][a, Q*256 + ql*8 + 4n + h] = idx16[64n+16h+a, Q*32+ql]
                    for n in range(NS):
                        for h in range(4):
                            src = idx_t[k][n * 64 + 16 * h : n * 64 + 16 * h + 16, :]
                            dst = bass.AP(
                                tensor=idxd[:].tensor,
                                offset=k * 16 * 1024 + 4 * n + h,
                                ap=[[1024, 16], [256, NQ], [8, QB]])
                            nc.sync.dma_start(dst, src)
                    # read back with 8x partition replication
                    for Q in range(NQ):
                        src = bass.AP(
                            tensor=idxd[:].tensor,
                            offset=k * 16 * 1024 + Q * 8 * QB,
                            ap=[[0, 8], [1024, 16], [1, 8 * QB]])
                        nc.sync.dma_start(wkq_t[k][Q][:], src)
                    if debug:
                        nc.sync.dma_start(d_idx[k], idx_t[k][:])
                        nc.sync.dma_start(d_alpha[k], alpha_t[k][:])
                if debug:
                    for n in range(NS):
                        nc.sync.dma_start(d_offs[n], offs_tiles[n][:])

            # ---------------- phase 2 ----------------
            if stage == 'p1':
                continue
            with tc.tile_pool(name=f"gb{rep}", bufs=3) as gbp, \
                 tc.tile_pool(name=f"dd{rep}", bufs=2) as ddp, \
                 tc.tile_pool(name=f"ee{rep}", bufs=2) as eep, \
                 tc.tile_pool(name=f"outst{rep}", bufs=2) as outp, \
                 tc.tile_pool(name=f"mpsum{rep}", bufs=1, space="PSUM") as mpsump:
                for Q in range(NQ):
                    ps = [mpsump.tile([CO, QW], f32, tag=f"ps{n}", name=f"ps{rep}_{n}_{Q}") for n in range(NS)] \
                        if stage != 'nomm' else None
                    for k in range(K):
                        gt = gbp.tile([128, QB, 128], f32, tag="gb")
                        xt2_pair = bass.AP(tensor=xt2[:].tensor, offset=0,
                                           ap=[[64, NS * XROWS - 1], [1, 128]])
                        nc.gpsimd.dma_gather(
                            gt[:], xt2_pair,
                            wkq_t[k][Q][:],
                            num_idxs=128 * QB, num_idxs_reg=128 * QB,
                            elem_size=128, elem_step=64, single_packet=False,
                        )
                        if debug and Q == 0 and k == 0:
                            nc.sync.dma_start(d_gb[:], gt[:])
                        if stage == 'nomm':
                            continue
                        dt_ = ddp.tile([128, QB, 64], f32, tag="dd")
                        nc.vector.tensor_tensor(dt_[:], gt[:, :, 64:128],
                                                gt[:, :, 0:64], op=Alu.subtract)
                        et = eep.tile([128, QB, 64], f32, tag="ee")
                        nc.vector.tensor_tensor(
                            et[:], dt_[:],
                            alpha_t[k][:, Q * QB : (Q + 1) * QB]
                            .to_broadcast([128, QB, 64]),
                            op=Alu.mult)
                        for n in range(NS):
                            lhs = wTk2_t[n * 64 : (n + 1) * 64, k * 64 : (k + 1) * 64]
                            for ridx, rhs_full in ((0, gt[n * 64 : (n + 1) * 64, :, 0:64]),
                                                   (1, et[n * 64 : (n + 1) * 64, :, :])):
                                for b in range(QW // 512):
                                    nc.tensor.matmul(
                                        ps[n][:, b * 512 : (b + 1) * 512],
                                        lhsT=lhs,
                                        rhs=rhs_full[:, b * 8 : (b + 1) * 8, :],
                                        start=(k == 0 and ridx == 0),
                                        stop=(k == K - 1 and ridx == 1),
                                    )
                    for n in range(NS if stage != 'nomm' else 0):
                        ot = outp.tile([CO, QW], f32, tag=f"ot{n}")
                        nc.scalar.activation(ot[:], ps[n][:], Act.Identity,
                                             bias=bout_t[:])
                        nc.sync.dma_start(out[n, :, Q * QW : (Q + 1) * QW], ot[:])

    nc.compile()
    return nc


def _host_tables(w_off, w, b_off, b):
    woffT = np.ascontiguousarray(
        w_off[[0, 2, 4], :, :].transpose(1, 2, 0).reshape(C, 9)).astype(np.float32)
    wTk = np.ascontiguousarray(
        w.reshape(CO, K, 64).transpose(2, 1, 0).reshape(64, K * CO)).astype(np.float32)
    wTk2 = np.concatenate([wTk, wTk], axis=0)
    p = np.arange(128)[:, None]
    q = np.arange(128)[None, :]
    base128 = ((p % 64) * 128 + q + 1).astype(np.float32)
    row128 = (XROWS * (p // 64) + 0 * q).astype(np.float32)
    boff3 = np.ascontiguousarray(b_off[[0, 2, 4]].reshape(3, 1)).astype(np.float32)
    bout = np.ascontiguousarray(b.reshape(CO, 1)).astype(np.float32)
    ident = np.eye(64, dtype=np.float32)
    return dict(woffT=woffT, wTk2=wTk2, base128=base128, row128=row128,
                boff=boff3, bout=bout, ident=ident)


def get_nc(debug=False, reps=1, stage='full'):
    key = f"nc_{int(debug)}_{reps}_{stage}"
    if key not in _CACHE:
        _CACHE[key] = _build_nc(debug, reps, stage)
    return _CACHE[key]


def _get_callable(debug=False, reps=1, stage='full'):
    """Jitted 8-core shard_map program running the NEFF; compiled once."""
    fkey = f"fn_{int(debug)}_{reps}_{stage}"
    if fkey in _CACHE:
        return _CACHE[fkey]
    import jax
    from jax.sharding import Mesh, PartitionSpec
    from jax.experimental.shard_map import shard_map
    from concourse import bass2jax, mybir

    bass2jax.install_neuronx_cc_hook()
    nc = get_nc(debug, reps, stage)
    partition_name = nc.partition_id_tensor.name if nc.partition_id_tensor else None
    in_names, out_names, out_avals = [], [], []
    for alloc in nc.m.functions[0].allocations:
        if not isinstance(alloc, mybir.MemoryLocationSet):
            continue
        name = alloc.memorylocations[0].name
        if alloc.kind == "ExternalInput":
            if name != partition_name:
                in_names.append(name)
        elif alloc.kind == "ExternalOutput":
            out_names.append(name)
            out_avals.append(jax.core.ShapedArray(
                tuple(alloc.tensor_shape), mybir.dt.np(alloc.dtype)))
    n_params = len(in_names)
    all_in_names = list(in_names) + list(out_names)
    if partition_name is not None:
        all_in_names.append(partition_name)

    def _body(*args):
        operands = list(args)
        if partition_name is not None:
            operands.append(bass2jax.partition_id_tensor())
        outs = bass2jax._bass_exec_p.bind(
            *operands,
            out_avals=tuple(out_avals),
            in_names=tuple(all_in_names),
            out_names=tuple(out_names),
            lowering_input_output_aliases=(),
            sim_require_finite=True,
            sim_require_nnan=True,
            nc=nc,
        )
        return tuple(outs)

    devices = jax.devices()[:NCORES]
    mesh = Mesh(np.asarray(devices), ("core",))
    n_all = n_params + len(out_names)
    sharded = jax.jit(
        shard_map(_body, mesh=mesh,
                  in_specs=(PartitionSpec("core"),) * n_all,
                  out_specs=(PartitionSpec("core"),) * len(out_names),
                  check_rep=False),
        keep_unused=True,
    )
    _CACHE[fkey] = (sharded, in_names, out_names, out_avals, mesh)
    return _CACHE[fkey]


def _concat_inputs(x, w_off, b_off, w, b, in_names, out_avals):
    tables = _host_tables(np.asarray(w_off), np.asarray(w),
                          np.asarray(b_off), np.asarray(b))
    x = np.ascontiguousarray(np.asarray(x), dtype=np.float32)
    per_core = []
    for i in range(NCORES):
        m = dict(tables)
        m["xin"] = np.ascontiguousarray(x[i * NS:(i + 1) * NS])
        per_core.append(m)
    concat = [np.concatenate([per_core[c][nm] for c in range(NCORES)], axis=0)
              for nm in in_names]
    zeros = [np.zeros((NCORES * av.shape[0], *av.shape[1:]), av.dtype)
             for av in out_avals]
    return concat + zeros


def kernel(x, w_off, b_off, w, b):
    fn, in_names, out_names, out_avals, mesh = _get_callable()
    args = _concat_inputs(x, w_off, b_off, w, b, in_names, out_avals)
    outs = fn(*args)
    oidx = out_names.index("out")
    full = np.asarray(outs[oidx]).reshape(NCORES * NS, CO, L).astype(np.float32)
    return full


def timeit(x, w_off, b_off, w, b, iters=30, reps=1, stage='full'):
    import time
    import jax
    from jax.sharding import NamedSharding, PartitionSpec
    fn, in_names, out_names, out_avals, mesh = _get_callable(reps=reps, stage=stage)
    args = _concat_inputs(x, w_off, b_off, w, b, in_names, out_avals)
    sh = NamedSharding(mesh, PartitionSpec("core"))
    dargs = [jax.device_put(a, sh) for a in args]
    outs = fn(*dargs)
    jax.block_until_ready(outs)
    t0 = time.perf_counter()
    for _ in range(iters):
        outs = fn(*dargs)
    jax.block_until_ready(outs)
    t1 = time.perf_counter()
    return (t1 - t0) / iters * 1e9

